# revision 1
# baseline (speedup 1.0000x reference)
"""Single-head causal attention with RoPE on 8 TRN2 NeuronCores.

Sharding: core c -> batch c//2, parity p = c%2 takes the interleaved
512-row q-blocks {p, p+2, p+4, p+6} of T=4096 (causal load balance).
Each core computes full K/V for its batch (duplicated across the pair),
so no collectives are needed.

Device layout tricks:
- xT passed host-transposed and column-permuted into "slot" order
  [own q-blocks | other blocks] so the SPMD program is identical on all
  cores (q projection always for t-slots 0..15).
- Wq/Wk rows host-permuted evens-first so RoPE becomes rotate-half form
  (free-dim ops only); scores are permutation-invariant.
- Scores computed transposed (S^T[s, q]) so softmax P^T feeds the AV
  matmul directly; row sums via ones-vector matmuls; causal masking via
  exp bias (-1e9) for the data-dependent tail block plus a static
  triangular multiplicative mask for the diagonal block.
"""
import numpy as np

B, T, C, HD = 4, 4096, 2048, 128
P = 128
NB = 8          # 512-row blocks per sequence
BS = 512        # block size
SCALE = float(C) ** -0.5
NEG = -1.0e9


def build():
    import concourse.bass as bass
    import concourse.mybir as mybir
    import bass_rust
    from concourse.tile import TileContext
    from concourse.masks import make_identity

    f32 = mybir.dt.float32
    f32r = mybir.dt.float32r
    EXP = mybir.ActivationFunctionType.Exp

    nc = bass.Bass()
    xt = nc.declare_dram_parameter("xt", [C, T], f32, isOutput=False)
    w = nc.declare_dram_parameter("w", [C, 3 * HD], f32, isOutput=False)
    cos2 = nc.declare_dram_parameter("cos2", [T, P], f32, isOutput=False)
    sin2 = nc.declare_dram_parameter("sin2", [T, P], f32, isOutput=False)
    tailb = nc.declare_dram_parameter("tailb", [P, 1], f32, isOutput=False)
    out = nc.declare_dram_parameter("out", [T // 2, HD], f32, isOutput=True)

    xtr = xt.bitcast(f32r)
    wr = w.bitcast(f32r)

    with TileContext(nc) as tc:
        with (
            tc.tile_pool(name="const", bufs=1) as cp,
            tc.tile_pool(name="xp", bufs=2) as xp,
            tc.tile_pool(name="rot", bufs=2) as rp,
            tc.tile_pool(name="pt", bufs=3) as ptp,
            tc.tile_pool(name="osb", bufs=2) as osb,
            tc.tile_pool(name="pps", bufs=2, space="PSUM") as pps,
            tc.tile_pool(name="tps", bufs=2, space="PSUM") as tps,
            tc.tile_pool(name="sps", bufs=2, space="PSUM") as sps,
            tc.tile_pool(name="o2ps", bufs=1, space="PSUM") as o2ps,
            tc.tile_pool(name="smps", bufs=1, space="PSUM") as smps,
        ):
            # ---- constants / resident tensors ----
            ident = cp.tile([P, P], f32, tag="ident")
            make_identity(nc, ident[:])
            ones = cp.tile([P, 2], f32, tag="ones")
            nc.gpsimd.memset(ones[:], 1.0)
            tri = cp.tile([P, 4 * BS], f32, tag="tri")
            nc.gpsimd.memset(tri[:], 0.0)
            for j in range(4):
                # tri_j[s, q] = 1.0 where s + 128*j <= q else 0.0
                nc.gpsimd.affine_select(
                    out=tri[:, j * BS:(j + 1) * BS],
                    in_=tri[:, j * BS:(j + 1) * BS],
                    compare_op=mybir.AluOpType.is_gt,
                    fill=1.0, base=j * P,
                    pattern=[[-1, BS]], channel_multiplier=1,
                )
            wt = cp.tile([P, 16 * 384], f32r, tag="wt")
            for g in range(4):   # 4 DMAs -> 4 queues
                nc.sync.dma_start(
                    wt[:, g * 4 * 384:(g + 1) * 4 * 384].rearrange(
                        "p (k n) -> p k n", k=4),
                    wr[g * 512:(g + 1) * 512, :].rearrange(
                        "(k p) n -> p k n", p=P))
            cst = cp.tile([P, 32 * P], f32, tag="cst")
            snt = cp.tile([P, 32 * P], f32, tag="snt")
            for g in range(4):
                sl = slice(g * 8 * P, (g + 1) * 8 * P)
                nc.sync.dma_start(
                    cst[:, sl].rearrange("p (k n) -> p k n", k=8),
                    cos2[g * 8 * P:(g + 1) * 8 * P, :].rearrange(
                        "(k p) n -> p k n", p=P))
                nc.sync.dma_start(
                    snt[:, sl].rearrange("p (k n) -> p k n", k=8),
                    sin2[g * 8 * P:(g + 1) * 8 * P, :].rearrange(
                        "(k p) n -> p k n", p=P))
            tb = cp.tile([P, 1], f32, tag="tb")
            nc.sync.dma_start(tb[:], tailb[:])

            qT = cp.tile([P, 16 * P], f32r, tag="qT")   # [d, 2048]
            kT = cp.tile([P, 32 * P], f32r, tag="kT")   # [d, 4096]
            vsb = cp.tile([P, 32 * P], f32r, tag="vsb")  # v[s,d] by s-tile

            # ---- phase 1: joint projection + RoPE + transposes ----
            for tg in range(8):          # t-groups of 512 (slot order)
                xts = []
                for ci in range(16):
                    xtile = xp.tile([P, BS], f32r, tag=f"x{ci}")
                    nc.sync.dma_start(
                        xtile[:], xtr[ci * P:(ci + 1) * P,
                                      tg * BS:(tg + 1) * BS])
                    xts.append(xtile)
                for sub in range(4):
                    t128 = tg * 4 + sub
                    nq = 384 if t128 < 16 else 256   # [k|v|q] layout
                    pp = pps.tile([P, 384], f32, tag="pp")
                    for ci in range(16):
                        nc.tensor.matmul(
                            pp[:, 0:nq],
                            xts[ci][:, sub * P:(sub + 1) * P],
                            wt[:, ci * 384:ci * 384 + nq],
                            start=(ci == 0), stop=(ci == 15))
                    cs = cst[:, t128 * P:(t128 + 1) * P]
                    sn = snt[:, t128 * P:(t128 + 1) * P]
                    H = 64

                    def rope(src_off, dst):
                        s0 = pp[:, src_off:src_off + P]
                        nc.vector.tensor_mul(dst[:], s0, cs)
                        tmp = rp.tile([P, P], f32, tag="ropetmp")
                        nc.vector.tensor_mul(
                            tmp[:, 0:H], pp[:, src_off + H:src_off + P],
                            sn[:, 0:H])
                        nc.vector.tensor_mul(
                            tmp[:, H:P], pp[:, src_off:src_off + H],
                            sn[:, H:P])
                        nc.vector.tensor_add(dst[:], dst[:], tmp[:])

                    rk = rp.tile([P, P], f32, tag="rk")
                    rope(0, rk)
                    nc.scalar.copy(vsb[:, t128 * P:(t128 + 1) * P],
                                   pp[:, P:2 * P])
                    tpk = tps.tile([P, P], f32, tag="tp")
                    nc.tensor.transpose(tpk[:], rk[:], ident[:])
                    nc.scalar.copy(kT[:, t128 * P:(t128 + 1) * P], tpk[:])
                    if t128 < 16:
                        rq = rp.tile([P, P], f32, tag="rq")
                        rope(2 * P, rq)
                        tpq = tps.tile([P, P], f32, tag="tp")
                        nc.tensor.transpose(tpq[:], rq[:], ident[:])
                        nc.scalar.copy(qT[:, t128 * P:(t128 + 1) * P],
                                       tpq[:])

            # ---- phase 2: attention per q-slot ----
            for j in range(4):
                qsl = slice(j * BS, (j + 1) * BS)
                o2 = o2ps.tile([P, BS], f32, tag="o2")
                sm = smps.tile([1, BS], f32, tag="sm")
                slots = ([(s, "full") for s in range(j)]
                         + [(4 + s, "full") for s in range(j)]
                         + [(j, "diag"), (4 + j, "tail")])
                nmm = len(slots) * 4
                mm = 0
                for (si, kind) in slots:
                    for st in range(4):
                        scol = si * BS + st * P
                        Sps = sps.tile([P, BS], f32, tag="S")
                        nc.tensor.matmul(Sps[:], kT[:, scol:scol + P],
                                         qT[:, qsl], start=True, stop=True)
                        Pt = ptp.tile([P, BS], f32r, tag="Pt")
                        bias = tb[:, 0:1] if kind == "tail" else 0.0
                        nc.scalar.activation(Pt[:], Sps[:], EXP,
                                             bias=bias, scale=SCALE)
                        if kind == "diag":
                            nc.vector.tensor_mul(
                                Pt[:], Pt[:], tri[:, st * BS:(st + 1) * BS])
                        nc.tensor.matmul(o2[:], vsb[:, scol:scol + P], Pt[:],
                                         start=(mm == 0), stop=(mm == nmm - 1))
                        nc.tensor.matmul(sm[:], ones[:, 0:1].bitcast(f32r), Pt[:],
                                         start=(mm == 0), stop=(mm == nmm - 1))
                        mm += 1
                # normalize + transpose + store
                smsb = osb.tile([1, BS], f32, tag="smsb")
                nc.scalar.copy(smsb[:], sm[:])
                o2sb = osb.tile([P, BS], f32, tag="o2sb")
                nc.scalar.copy(o2sb[:], o2[:])
                rcp = osb.tile([P, 4], f32, tag="rcp")
                for ch in range(4):
                    rs = tps.tile([P, 1], f32, tag="tp")
                    nc.tensor.transpose(rs[:], smsb[0:1, ch * P:(ch + 1) * P],
                                        ident[0:1, 0:1])
                    nc.vector.reciprocal(rcp[:, ch:ch + 1], rs[:])
                for ch in range(4):
                    ot = tps.tile([P, P], f32, tag="tp")
                    nc.tensor.transpose(ot[:], o2sb[:, ch * P:(ch + 1) * P],
                                        ident[:])
                    osbt = osb.tile([P, P], f32, tag="ofin")
                    nc.vector.tensor_scalar_mul(osbt[:], ot[:],
                                                rcp[:, ch:ch + 1])
                    r0 = j * BS + ch * P
                    nc.sync.dma_start(out[r0:r0 + P, :], osbt[:])

    bass_rust.generate_event_semaphores(nc)
    return nc


_CACHE = {}


def _get_nc():
    if "nc" not in _CACHE:
        _CACHE["nc"] = build()
    return _CACHE["nc"]


def _prep_inputs(x, Wq, Wk, Wv, cos, sin):
    perm = np.concatenate([np.arange(0, HD, 2), np.arange(1, HD, 2)])
    wq = Wq[perm].astype(np.float32)
    wk = Wk[perm].astype(np.float32)
    w = np.concatenate([wk.T, Wv.T.astype(np.float32), wq.T], axis=1)
    w = np.ascontiguousarray(w)  # [C, 384] = [k|v|q]
    cos2 = np.concatenate([cos, cos], axis=1).astype(np.float32)
    sin2 = np.concatenate([-sin, sin], axis=1).astype(np.float32)
    in_maps = []
    orders = []
    for c in range(8):
        b, par = c // 2, c % 2
        order = [par, par + 2, par + 4, par + 6,
                 1 - par, 3 - par, 5 - par, 7 - par]
        orders.append(order)
        xb = np.asarray(x[b], np.float32)          # [T, C]
        xtp = np.empty((C, T), np.float32)
        c2 = np.empty((T, P), np.float32)
        s2 = np.empty((T, P), np.float32)
        for sl, ab in enumerate(order):
            dst = slice(sl * BS, (sl + 1) * BS)
            src = slice(ab * BS, (ab + 1) * BS)
            xtp[:, dst] = xb[src].T
            c2[dst] = cos2[src]
            s2[dst] = sin2[src]
        tailb = np.full((P, 1), NEG if par == 0 else 0.0, np.float32)
        in_maps.append({"xt": np.ascontiguousarray(xtp), "w": w,
                        "cos2": np.ascontiguousarray(c2),
                        "sin2": np.ascontiguousarray(s2), "tailb": tailb})
    return in_maps, orders


def _run(x, Wq, Wk, Wv, cos, sin, trace=False):
    from concourse.bass_utils import run_bass_kernel_spmd
    nc = _get_nc()
    in_maps, orders = _prep_inputs(x, Wq, Wk, Wv, cos, sin)
    res = run_bass_kernel_spmd(nc, in_maps, list(range(8)), trace=trace)
    full = np.empty((B, T, HD), np.float32)
    for c in range(8):
        b, order = c // 2, orders[c]
        oc = res.results[c]["out"]
        for j in range(4):
            ab = order[j]
            full[b, ab * BS:(ab + 1) * BS] = oc[j * BS:(j + 1) * BS]
    return full, res


def kernel(x, Wq, Wk, Wv, cos, sin):
    return _run(x, Wq, Wk, Wv, cos, sin, trace=False)[0]



# revision 9
# speedup vs baseline: 1.1382x; 1.1382x over previous
"""Single-head causal attention with RoPE on 8 TRN2 NeuronCores.

Sharding: core c -> batch c//2, parity p = c%2 takes the interleaved
512-row q-blocks {p, p+2, p+4, p+6} of T=4096. Each core projects
q/k/v only for its OWN 2048 rows (bf16 matmuls); the pair exchanges
exact f32 K/V via an AllReduce(add) on a DRAM bounce buffer and
recovers the partner's half as (sum - own) on the vector engine.

Attention computes transposed scores S^T[s, q] so softmax feeds the
PE directly; the AV matmul uses P^T chunks as the stationary operand
with a [v | ones] moving operand, so each 128-q output chunk lands
row-major in PSUM with its softmax denominator in column 128 - no
output transposes and no separate row-sum matmuls. Causal masking:
static triangular mask on the 128x128 diagonal sub-blocks (with the
q-range of diagonal 512-blocks trimmed per s-subtile) plus a
data-dependent exp bias (-1e9) for the partner tail block.
"""
import numpy as np
import ml_dtypes

B, T, C, HD = 4, 4096, 2048, 128
P = 128
BS = 512
T2 = T // 2          # own rows per core
NT = T2 // P         # 16 own 128-blocks
SCALE = float(C) ** -0.5
NEG = -1.0e9
bf16 = ml_dtypes.bfloat16


def build():
    import concourse.bass as bass
    import concourse.mybir as mybir
    import bass_rust
    from concourse.tile import TileContext
    from concourse.masks import make_identity

    f32 = mybir.dt.float32
    bf = mybir.dt.bfloat16
    EXP = mybir.ActivationFunctionType.Exp

    nc = bass.Bass(num_devices=8)
    xt = nc.declare_dram_parameter("xt", [C, T2], bf, isOutput=False)
    w = nc.declare_dram_parameter("w", [C, 3 * HD], bf, isOutput=False)
    cos2 = nc.declare_dram_parameter("cos2", [T2, P], bf, isOutput=False)
    sin2 = nc.declare_dram_parameter("sin2", [T2, P], bf, isOutput=False)
    tailb = nc.declare_dram_parameter("tailb", [P, 1], f32, isOutput=False)
    out = nc.declare_dram_parameter("out", [T2, HD], f32, isOutput=True)

    VSTR = 132           # v block stride in vs ([128 v | 1 ones | 3 pad])
    H = 64

    with TileContext(nc) as tc:
        with (
            tc.tile_pool(name="const", bufs=1) as cp,
            tc.tile_pool(name="xp", bufs=2) as xp,
            tc.tile_pool(name="rot", bufs=2) as rp,
            tc.tile_pool(name="pt", bufs=3) as ptp,
            tc.tile_pool(name="osb", bufs=2) as osb,
            tc.tile_pool(name="rec", bufs=2) as rec,
            tc.tile_pool(name="dram", bufs=1, space="DRAM") as dram,
            tc.tile_pool(name="pps", bufs=2, space="PSUM") as pps,
            tc.tile_pool(name="tps", bufs=2, space="PSUM") as tps,
            tc.tile_pool(name="sps", bufs=2, space="PSUM") as sps,
            tc.tile_pool(name="ops", bufs=1, space="PSUM") as ops,
        ):
            # ---- constants / resident tensors ----
            ident = cp.tile([P, P], f32, tag="ident")
            make_identity(nc, ident[:])
            trif = cp.tile([P, P], f32, tag="trif")
            nc.gpsimd.memset(trif[:], 0.0)
            # tri[s, q] = 1.0 where s <= q
            nc.gpsimd.affine_select(
                out=trif[:], in_=trif[:],
                compare_op=mybir.AluOpType.is_gt,
                fill=1.0, base=0,
                pattern=[[-1, P]], channel_multiplier=1,
            )
            tri = cp.tile([P, P], bf, tag="tri")
            nc.vector.tensor_copy(tri[:], trif[:])
            wt = cp.tile([P, 16 * 384], bf, tag="wt")
            for g in range(4):
                nc.sync.dma_start(
                    wt[:, g * 4 * 384:(g + 1) * 4 * 384].rearrange(
                        "p (k n) -> p k n", k=4),
                    w[g * 512:(g + 1) * 512, :].rearrange(
                        "(k p) n -> p k n", p=P))
            cst = cp.tile([P, NT * P], bf, tag="cst")
            snt = cp.tile([P, NT * P], bf, tag="snt")
            for g in range(2):
                sl = slice(g * 8 * P, (g + 1) * 8 * P)
                nc.sync.dma_start(
                    cst[:, sl].rearrange("p (k n) -> p k n", k=8),
                    cos2[g * 8 * P:(g + 1) * 8 * P, :].rearrange(
                        "(k p) n -> p k n", p=P))
                nc.sync.dma_start(
                    snt[:, sl].rearrange("p (k n) -> p k n", k=8),
                    sin2[g * 8 * P:(g + 1) * 8 * P, :].rearrange(
                        "(k p) n -> p k n", p=P))
            tb = cp.tile([P, 1], f32, tag="tb")
            nc.sync.dma_start(tb[:], tailb[:])

            qT = cp.tile([P, NT * P], bf, tag="qT")      # [d, 2048] own q^T
            kT = cp.tile([P, 32 * P], bf, tag="kT")      # [d, 4096] own|partner
            vs = cp.tile([P, 32 * VSTR], bf, tag="vs")   # v rows + ones col
            nc.gpsimd.memset(vs[:], 0.0)
            # ones column at offset 128 of every block
            for i in range(32):
                nc.gpsimd.memset(vs[:, i * VSTR + P:i * VSTR + P + 1], 1.0)

            # DRAM bounce: half h holds [k blocks | v blocks] of 8 t-blocks
            cin = [dram.tile([P, 16 * P], f32, name=f"cin{h}")
                   for h in range(2)]
            cout = [dram.tile([P, 16 * P], f32, name=f"cout{h}")
                    for h in range(2)]

            # ---- phase 1: projection + RoPE (own 2048 rows) ----
            for tg in range(4):          # 512-row groups
                xts = []
                for ci in range(16):
                    xtile = xp.tile([P, BS], bf, tag=f"x{ci}")
                    nc.sync.dma_start(
                        xtile[:], xt[ci * P:(ci + 1) * P,
                                     tg * BS:(tg + 1) * BS])
                    xts.append(xtile)
                for sub in range(4):
                    t128 = tg * 4 + sub
                    h, hb = t128 // 8, t128 % 8
                    pp = pps.tile([P, 384], f32, tag="pp")
                    for ci in range(16):
                        nc.tensor.matmul(
                            pp[:],
                            xts[ci][:, sub * P:(sub + 1) * P],
                            wt[:, ci * 384:(ci + 1) * 384],
                            start=(ci == 0), stop=(ci == 15))
                    cs = cst[:, t128 * P:(t128 + 1) * P]
                    sn = snt[:, t128 * P:(t128 + 1) * P]

                    def rope(src_off, dst):
                        s0 = pp[:, src_off:src_off + P]
                        nc.vector.tensor_mul(dst[:], s0, cs)
                        tmp = rp.tile([P, P], f32, tag="ropetmp")
                        nc.vector.tensor_mul(
                            tmp[:, 0:H], pp[:, src_off + H:src_off + P],
                            sn[:, 0:H])
                        nc.vector.tensor_mul(
                            tmp[:, H:P], pp[:, src_off:src_off + H],
                            sn[:, H:P])
                        nc.vector.tensor_add(dst[:], dst[:], tmp[:])

                    rk = rp.tile([P, P], f32, tag="rk")
                    rope(0, rk)
                    tpk = tps.tile([P, P], f32, tag="tp")
                    nc.tensor.transpose(tpk[:], rk[:], ident[:])
                    kst = rp.tile([P, P], f32, tag="kst")
                    nc.vector.tensor_copy(kst[:], tpk[:])
                    nc.scalar.copy(kT[:, t128 * P:(t128 + 1) * P], kst[:])
                    nc.sync.dma_start(cin[h][:, hb * P:(hb + 1) * P], kst[:])
                    # v block: row layout
                    vst = rp.tile([P, P], f32, tag="vst")
                    nc.vector.tensor_copy(vst[:], pp[:, P:2 * P])
                    nc.scalar.copy(vs[:, t128 * VSTR:t128 * VSTR + P], vst[:])
                    nc.sync.dma_start(cin[h][:, (8 + hb) * P:(9 + hb) * P],
                                      vst[:])
                    rq = rp.tile([P, P], f32, tag="rq")
                    rope(2 * P, rq)
                    tpq = tps.tile([P, P], f32, tag="tp")
                    nc.tensor.transpose(tpq[:], rq[:], ident[:])
                    nc.scalar.copy(qT[:, t128 * P:(t128 + 1) * P], tpq[:])

                # after first/second half of blocks: fire the collective
                if tg == 1 or tg == 3:
                    h = tg // 2
                    nc.gpsimd.collective_compute(
                        "AllReduce",
                        mybir.AluOpType.add,
                        replica_groups=[[0, 1], [2, 3], [4, 5], [6, 7]],
                        ins=[cin[h].opt()],
                        outs=[cout[h].opt()],
                    )
                    ksum = rec.tile([P, 8 * P], f32, tag="ksum")
                    vsum = rec.tile([P, 8 * P], f32, tag="vsum")
                    kown = rec.tile([P, 8 * P], f32, tag="kown")
                    vown = rec.tile([P, 8 * P], f32, tag="vown")
                    nc.sync.dma_start(ksum[:], cout[h][:, 0:8 * P])
                    nc.sync.dma_start(vsum[:], cout[h][:, 8 * P:16 * P])
                    nc.sync.dma_start(kown[:], cin[h][:, 0:8 * P])
                    nc.sync.dma_start(vown[:], cin[h][:, 8 * P:16 * P])
                    nc.vector.tensor_sub(
                        kT[:, (16 + 8 * h) * P:(24 + 8 * h) * P],
                        ksum[:], kown[:])
                    for i in range(8):
                        blk = 16 + 8 * h + i
                        nc.vector.tensor_sub(
                            vs[:, blk * VSTR:blk * VSTR + P],
                            vsum[:, i * P:(i + 1) * P],
                            vown[:, i * P:(i + 1) * P])

            # ---- phase 2: attention per q-slot ----
            for j in range(4):
                o = [ops.tile([P, 2 * 129], f32, tag=f"o{m}", name=f"o{m}")
                     for m in range(2)]
                # start=True zeroes the whole PSUM bank, which would wipe the
                # sibling chunk's chain sharing the bank - zero once instead.
                nc.vector.memset(o[0][:], 0.0)
                nc.vector.memset(o[1][:], 0.0)
                nav = [0] * 4        # AV matmuls already emitted per q-chunk
                tot = [8 * j + qc + 5 for qc in range(4)]
                # (own fulls, diag, partner fulls, partner tail)
                slots = ([("own", si, "full") for si in range(j)]
                         + [("own", j, "diag")]
                         + [("part", pi, "full") for pi in range(j)]
                         + [("part", j, "tail")])
                for (side, si, kind) in slots:
                    base = si * 4 if side == "own" else 16 + si * 4
                    for st in range(4):
                        blk = base + st                      # 128-block index
                        scol = blk * P
                        trim = st * P if kind == "diag" else 0
                        qlen = BS - trim
                        Sps = sps.tile([P, BS], f32, tag="S")
                        nc.tensor.matmul(
                            Sps[:, 0:qlen], kT[:, scol:scol + P],
                            qT[:, j * BS + trim:j * BS + BS],
                            start=True, stop=True)
                        Pt = ptp.tile([P, BS], bf, tag="Pt")
                        bias = tb[:, 0:1] if kind == "tail" else 0.0
                        nc.scalar.activation(Pt[:, 0:qlen], Sps[:, 0:qlen],
                                             EXP, bias=bias, scale=SCALE)
                        if kind == "diag":
                            nc.vector.tensor_mul(Pt[:, 0:P], Pt[:, 0:P],
                                                 tri[:])
                        nch = qlen // P
                        for ch in range(nch):
                            qc = trim // P + ch
                            om, oc = o[qc // 2], (qc % 2) * 129
                            nc.tensor.matmul(
                                om[:, oc:oc + 129],
                                Pt[:, ch * P:(ch + 1) * P],
                                vs[:, blk * VSTR:blk * VSTR + 129],
                                start=False,
                                stop=(nav[qc] == tot[qc] - 1),
                                skip_group_check=True)
                            nav[qc] += 1
                # normalize + store
                for qc in range(4):
                    om, oc = o[qc // 2], (qc % 2) * 129
                    rcp = osb.tile([P, 1], f32, tag="rcp")
                    nc.vector.reciprocal(rcp[:], om[:, oc + P:oc + P + 1])
                    ofin = osb.tile([P, P], f32, tag="ofin")
                    nc.vector.tensor_scalar_mul(ofin[:], om[:, oc:oc + P],
                                                rcp[:])
                    r0 = j * BS + qc * P
                    nc.sync.dma_start(out[r0:r0 + P, :], ofin[:])

    bass_rust.generate_event_semaphores(nc)
    return nc


_CACHE = {}


def _get_nc():
    if "nc" not in _CACHE:
        _CACHE["nc"] = build()
    return _CACHE["nc"]


def _prep_inputs(x, Wq, Wk, Wv, cos, sin):
    perm = np.concatenate([np.arange(0, HD, 2), np.arange(1, HD, 2)])
    wq = Wq[perm].astype(np.float32)
    wk = Wk[perm].astype(np.float32)
    w = np.concatenate([wk.T, Wv.T.astype(np.float32), wq.T], axis=1)
    w = np.ascontiguousarray(w).astype(bf16)   # [C, 384] = [k|v|q]
    cos2 = np.concatenate([cos, cos], axis=1).astype(np.float32)
    sin2 = np.concatenate([-sin, sin], axis=1).astype(np.float32)
    in_maps = []
    for c in range(8):
        b, par = c // 2, c % 2
        own = np.concatenate(
            [np.arange(a * BS, (a + 1) * BS) for a in (par, par + 2,
                                                       par + 4, par + 6)])
        xb = np.asarray(x[b], np.float32)
        xtp = np.ascontiguousarray(xb[own].T).astype(bf16)      # [C, T2]
        c2 = np.ascontiguousarray(cos2[own]).astype(bf16)
        s2 = np.ascontiguousarray(sin2[own]).astype(bf16)
        tb = np.full((P, 1), NEG if par == 0 else 0.0, np.float32)
        in_maps.append({"xt": xtp, "w": w, "cos2": c2, "sin2": s2,
                        "tailb": tb})
    return in_maps


def _run(x, Wq, Wk, Wv, cos, sin, trace=False):
    from concourse.bass_utils import run_bass_kernel_spmd
    nc = _get_nc()
    in_maps = _prep_inputs(x, Wq, Wk, Wv, cos, sin)
    res = run_bass_kernel_spmd(nc, in_maps, list(range(8)), trace=trace)
    full = np.empty((B, T, HD), np.float32)
    for c in range(8):
        b, par = c // 2, c % 2
        oc = res.results[c]["out"]
        for j in range(4):
            ab = par + 2 * j
            full[b, ab * BS:(ab + 1) * BS] = oc[j * BS:(j + 1) * BS]
    return full, res


def kernel(x, Wq, Wk, Wv, cos, sin):
    return _run(x, Wq, Wk, Wv, cos, sin, trace=False)[0]


# revision 13
# speedup vs baseline: 1.5204x; 1.3359x over previous
"""Single-head causal attention with RoPE on 8 TRN2 NeuronCores.

Sharding: core c -> batch c//2, parity p = c%2 takes the interleaved
512-row q-blocks {p, p+2, p+4, p+6} of T=4096. Each core projects
q/k/v only for its OWN 2048 rows (bf16 matmuls); pairs exchange bf16
K/V via four quarter-sized AllReduce(add) collectives on DRAM bounce
buffers (fired as each 512-row group finishes projecting, so they
hide under the rest of phase 1) and recover the partner's half as
(sum - own) on the vector engine.

Attention computes transposed scores S^T[s, q]; the AV matmul uses
P^T chunks as the stationary operand with a [v | ones] moving
operand, so each 128-q output chunk lands row-major in PSUM with its
softmax denominator in column 128 - no output transposes and no
separate row-sum matmuls. PSUM accumulation chains sharing a bank
are zeroed once via memset (matmul start=True zeroes the whole bank,
which would wipe the sibling chain). Causal masking: static
triangular mask on the diagonal 128x128 sub-blocks (with the q-range
of diagonal 512-blocks trimmed per s-subtile) plus a data-dependent
exp bias (-1e9) for the partner tail block.
"""
import numpy as np
import ml_dtypes

B, T, C, HD = 4, 4096, 2048, 128
P = 128
BS = 512
T2 = T // 2          # own rows per core
NT = T2 // P         # 16 own 128-blocks
SCALE = float(C) ** -0.5
NEG = -1.0e9
bf16 = ml_dtypes.bfloat16


def build():
    import concourse.bass as bass
    import concourse.mybir as mybir
    import bass_rust
    from concourse.tile import TileContext
    from concourse.masks import make_identity

    f32 = mybir.dt.float32
    bf = mybir.dt.bfloat16
    EXP = mybir.ActivationFunctionType.Exp

    nc = bass.Bass(num_devices=8)
    xt = nc.declare_dram_parameter("xt", [C, T2], bf, isOutput=False)
    w = nc.declare_dram_parameter("w", [C, 3 * HD], bf, isOutput=False)
    cos2 = nc.declare_dram_parameter("cos2", [T2, P], bf, isOutput=False)
    sin2 = nc.declare_dram_parameter("sin2", [T2, P], bf, isOutput=False)
    tailb = nc.declare_dram_parameter("tailb", [P, 1], f32, isOutput=False)
    out = nc.declare_dram_parameter("out", [T2, HD], f32, isOutput=True)

    VSTR = 132           # v block stride in vs ([128 v | 1 ones | 3 pad])
    H = 64

    with TileContext(nc) as tc:
        with (
            tc.tile_pool(name="const", bufs=1) as cp,
            tc.tile_pool(name="xp", bufs=2) as xp,
            tc.tile_pool(name="rot", bufs=2) as rp,
            tc.tile_pool(name="pt", bufs=3) as ptp,
            tc.tile_pool(name="osb", bufs=2) as osb,
            tc.tile_pool(name="rec", bufs=2) as rec,
            tc.tile_pool(name="dram", bufs=1, space="DRAM") as dram,
        ):
            # ---- constants / resident tensors ----
            identf = cp.tile([P, P], f32, tag="identf")
            make_identity(nc, identf[:])
            identb = cp.tile([P, P], bf, tag="identb")
            nc.vector.tensor_copy(identb[:], identf[:])
            trif = cp.tile([P, P], f32, tag="trif")
            nc.gpsimd.memset(trif[:], 0.0)
            # tri[s, q] = 1.0 where s <= q
            nc.gpsimd.affine_select(
                out=trif[:], in_=trif[:],
                compare_op=mybir.AluOpType.is_gt,
                fill=1.0, base=0,
                pattern=[[-1, P]], channel_multiplier=1,
            )
            tri = cp.tile([P, P], bf, tag="tri")
            nc.vector.tensor_copy(tri[:], trif[:])
            wt = cp.tile([P, 16 * 384], bf, tag="wt")
            for g in range(4):
                nc.sync.dma_start(
                    wt[:, g * 4 * 384:(g + 1) * 4 * 384].rearrange(
                        "p (k n) -> p k n", k=4),
                    w[g * 512:(g + 1) * 512, :].rearrange(
                        "(k p) n -> p k n", p=P))
            cst = cp.tile([P, NT * P], bf, tag="cst")
            snt = cp.tile([P, NT * P], bf, tag="snt")
            for g in range(2):
                sl = slice(g * 8 * P, (g + 1) * 8 * P)
                nc.sync.dma_start(
                    cst[:, sl].rearrange("p (k n) -> p k n", k=8),
                    cos2[g * 8 * P:(g + 1) * 8 * P, :].rearrange(
                        "(k p) n -> p k n", p=P))
                nc.sync.dma_start(
                    snt[:, sl].rearrange("p (k n) -> p k n", k=8),
                    sin2[g * 8 * P:(g + 1) * 8 * P, :].rearrange(
                        "(k p) n -> p k n", p=P))
            tb = cp.tile([P, 1], f32, tag="tb")
            nc.sync.dma_start(tb[:], tailb[:])

            qT = cp.tile([P, NT * P], bf, tag="qT")      # [d, 2048] own q^T
            kT = cp.tile([P, 32 * P], bf, tag="kT")      # [d, 4096] own|partner
            vs = cp.tile([P, 32 * VSTR], bf, tag="vs")   # v rows + ones col
            nc.gpsimd.memset(vs[:], 0.0)
            ones_ap = vs[:].rearrange("p (b c) -> p b c", b=32)[:, :, P:P + 1]
            nc.gpsimd.memset(ones_ap, 1.0)

            # DRAM bounce per quarter: [k blocks 4q..4q+3 | v blocks ...]
            cin = [dram.tile([P, 8 * P], bf, name=f"cin{q}")
                   for q in range(4)]
            cout = [dram.tile([P, 8 * P], bf, name=f"cout{q}")
                    for q in range(4)]

            # ---- phase 1: projection + RoPE (own 2048 rows) ----
            with tc.tile_pool(name="pps", bufs=2, space="PSUM") as pps, \
                 tc.tile_pool(name="tps", bufs=2, space="PSUM") as tps:
                for tg in range(4):
                    xbig = xp.tile([P, 16 * BS], bf, tag="x")
                    nc.sync.dma_start(
                        xbig[:].rearrange("p (k n) -> p k n", k=16),
                        xt[:, tg * BS:(tg + 1) * BS].rearrange(
                            "(k p) n -> p k n", p=P))
                    for sub in range(4):
                        t128 = tg * 4 + sub
                        pp = pps.tile([P, 384], f32, tag="pp")
                        for ci in range(16):
                            nc.tensor.matmul(
                                pp[:],
                                xbig[:, ci * BS + sub * P:
                                     ci * BS + (sub + 1) * P],
                                wt[:, ci * 384:(ci + 1) * 384],
                                start=(ci == 0), stop=(ci == 15))
                        cs = cst[:, t128 * P:(t128 + 1) * P]
                        sn = snt[:, t128 * P:(t128 + 1) * P]

                        def rope(src_off, dst):
                            s0 = pp[:, src_off:src_off + P]
                            nc.vector.tensor_mul(dst[:], s0, cs)
                            tmp = rp.tile([P, P], bf, tag="ropetmp")
                            nc.vector.tensor_mul(
                                tmp[:, 0:H], pp[:, src_off + H:src_off + P],
                                sn[:, 0:H])
                            nc.vector.tensor_mul(
                                tmp[:, H:P], pp[:, src_off:src_off + H],
                                sn[:, H:P])
                            nc.vector.tensor_add(dst[:], dst[:], tmp[:])

                        rk = rp.tile([P, P], bf, tag="rk")
                        rope(0, rk)
                        tpk = tps.tile([P, P], bf, tag="tp")
                        nc.tensor.transpose(tpk[:], rk[:], identb[:])
                        nc.scalar.copy(kT[:, t128 * P:(t128 + 1) * P], tpk[:])
                        nc.scalar.copy(vs[:, t128 * VSTR:t128 * VSTR + P],
                                       pp[:, P:2 * P])
                        rq = rp.tile([P, P], bf, tag="rq")
                        rope(2 * P, rq)
                        tpq = tps.tile([P, P], bf, tag="tp")
                        nc.tensor.transpose(tpq[:], rq[:], identb[:])
                        nc.vector.tensor_copy(qT[:, t128 * P:(t128 + 1) * P],
                                              tpq[:])

                    # quarter exchange: k and v blocks 4tg..4tg+3
                    q4 = tg
                    nc.sync.dma_start(cin[q4][:, 0:4 * P],
                                      kT[:, q4 * 4 * P:(q4 + 1) * 4 * P])
                    vsrc = vs[:, q4 * 4 * VSTR:(q4 + 1) * 4 * VSTR].rearrange(
                        "p (b c) -> p b c", b=4)[:, :, 0:P]
                    nc.sync.dma_start(
                        cin[q4][:, 4 * P:8 * P].rearrange(
                            "p (b c) -> p b c", b=4), vsrc)
                    nc.gpsimd.collective_compute(
                        "AllReduce", mybir.AluOpType.add,
                        replica_groups=[[0, 1], [2, 3], [4, 5], [6, 7]],
                        ins=[cin[q4].opt()], outs=[cout[q4].opt()],
                    )
                    ksum = rec.tile([P, 4 * P], bf, tag="ksum")
                    vsum = rec.tile([P, 4 * P], bf, tag="vsum")
                    nc.sync.dma_start(ksum[:], cout[q4][:, 0:4 * P])
                    nc.sync.dma_start(vsum[:], cout[q4][:, 4 * P:8 * P])
                    nc.vector.tensor_sub(
                        kT[:, (16 + 4 * q4) * P:(20 + 4 * q4) * P],
                        ksum[:], kT[:, q4 * 4 * P:(q4 + 1) * 4 * P])
                    for i in range(4):
                        blk = 16 + 4 * q4 + i
                        own = 4 * q4 + i
                        nc.vector.tensor_sub(
                            vs[:, blk * VSTR:blk * VSTR + P],
                            vsum[:, i * P:(i + 1) * P],
                            vs[:, own * VSTR:own * VSTR + P])

            # ---- phase 2: attention per q-slot ----
            with tc.tile_pool(name="sps", bufs=2, space="PSUM") as sps, \
                 tc.tile_pool(name="ops", bufs=2, space="PSUM") as ops:
                for j in range(4):
                    o = [ops.tile([P, 2 * 129], f32, tag=f"o{m}",
                                  name=f"o{m}") for m in range(2)]
                    nc.vector.memset(o[0][:], 0.0)
                    nc.vector.memset(o[1][:], 0.0)
                    nav = [0] * 4
                    tot = [8 * j + qc + 5 for qc in range(4)]
                    slots = ([("own", si, "full") for si in range(j)]
                             + [("own", j, "diag")]
                             + [("part", pi, "full") for pi in range(j)]
                             + [("part", j, "tail")])
                    for (side, si, kind) in slots:
                        base = si * 4 if side == "own" else 16 + si * 4
                        for sp in range(2):      # pairs (st, st+1)
                            sts = (2 * sp, 2 * sp + 1)
                            Sps = sps.tile([P, 2 * BS], f32, tag="S")
                            Pt = ptp.tile([P, 2 * BS], bf, tag="Pt")
                            offs = []
                            off = 0
                            for st in sts:
                                blk = base + st
                                trim = st * P if kind == "diag" else 0
                                qlen = BS - trim
                                nc.tensor.matmul(
                                    Sps[:, off:off + qlen],
                                    kT[:, blk * P:(blk + 1) * P],
                                    qT[:, j * BS + trim:j * BS + BS],
                                    start=True, stop=True)
                                offs.append((st, blk, trim, qlen, off))
                                off += qlen
                            bias = tb[:, 0:1] if kind == "tail" else 0.0
                            nc.scalar.activation(Pt[:, 0:off], Sps[:, 0:off],
                                                 EXP, bias=bias, scale=SCALE)
                            for (st, blk, trim, qlen, o0) in offs:
                                if kind == "diag":
                                    nc.vector.tensor_mul(
                                        Pt[:, o0:o0 + P], Pt[:, o0:o0 + P],
                                        tri[:])
                                for ch in range(qlen // P):
                                    qc = trim // P + ch
                                    om, oc = o[qc // 2], (qc % 2) * 129
                                    nc.tensor.matmul(
                                        om[:, oc:oc + 129],
                                        Pt[:, o0 + ch * P:o0 + (ch + 1) * P],
                                        vs[:, blk * VSTR:blk * VSTR + 129],
                                        start=False,
                                        stop=(nav[qc] == tot[qc] - 1),
                                        skip_group_check=True)
                                    nav[qc] += 1
                    # normalize + store
                    obig = osb.tile([P, 4 * P], f32, tag="obig")
                    for qc in range(4):
                        om, oc = o[qc // 2], (qc % 2) * 129
                        rcp = osb.tile([P, 1], f32, tag="rcp")
                        nc.vector.reciprocal(rcp[:], om[:, oc + P:oc + P + 1])
                        nc.vector.tensor_scalar_mul(
                            obig[:, qc * P:(qc + 1) * P], om[:, oc:oc + P],
                            rcp[:])
                    nc.sync.dma_start(
                        out[j * BS:(j + 1) * BS, :].rearrange(
                            "(k p) n -> p k n", p=P),
                        obig[:].rearrange("p (k n) -> p k n", k=4))

    bass_rust.generate_event_semaphores(nc)
    return nc


_CACHE = {}


def _get_nc():
    if "nc" not in _CACHE:
        _CACHE["nc"] = build()
    return _CACHE["nc"]


def _prep_inputs(x, Wq, Wk, Wv, cos, sin):
    perm = np.concatenate([np.arange(0, HD, 2), np.arange(1, HD, 2)])
    wq = Wq[perm].astype(np.float32)
    wk = Wk[perm].astype(np.float32)
    w = np.concatenate([wk.T, Wv.T.astype(np.float32), wq.T], axis=1)
    w = np.ascontiguousarray(w).astype(bf16)   # [C, 384] = [k|v|q]
    cos2 = np.concatenate([cos, cos], axis=1).astype(np.float32)
    sin2 = np.concatenate([-sin, sin], axis=1).astype(np.float32)
    in_maps = []
    for c in range(8):
        b, par = c // 2, c % 2
        own = np.concatenate(
            [np.arange(a * BS, (a + 1) * BS) for a in (par, par + 2,
                                                       par + 4, par + 6)])
        xb = np.asarray(x[b], np.float32)
        xtp = np.ascontiguousarray(xb[own].T).astype(bf16)      # [C, T2]
        c2 = np.ascontiguousarray(cos2[own]).astype(bf16)
        s2 = np.ascontiguousarray(sin2[own]).astype(bf16)
        tb = np.full((P, 1), NEG if par == 0 else 0.0, np.float32)
        in_maps.append({"xt": xtp, "w": w, "cos2": c2, "sin2": s2,
                        "tailb": tb})
    return in_maps


def _run(x, Wq, Wk, Wv, cos, sin, trace=False):
    from concourse.bass_utils import run_bass_kernel_spmd
    nc = _get_nc()
    in_maps = _prep_inputs(x, Wq, Wk, Wv, cos, sin)
    res = run_bass_kernel_spmd(nc, in_maps, list(range(8)), trace=trace)
    full = np.empty((B, T, HD), np.float32)
    for c in range(8):
        b, par = c // 2, c % 2
        oc = res.results[c]["out"]
        for j in range(4):
            ab = par + 2 * j
            full[b, ab * BS:(ab + 1) * BS] = oc[j * BS:(j + 1) * BS]
    return full, res


def kernel(x, Wq, Wk, Wv, cos, sin):
    return _run(x, Wq, Wk, Wv, cos, sin, trace=False)[0]
